# revision 49
# baseline (speedup 1.0000x reference)
"""Trainium2 Bass kernel for per-edge dot products (GNN DotPredictor).

out[e] = sum(h[src[e]] * h[dst[e]]); 800k edges, h [50k, 64] f32, 8 cores.

Design (v7):
  - Edges sharded 8 ways; h replicated. Per-edge rows fetched from HBM with
    the Q7 `dma_gather` path. The Q7 descriptor generation (~8ns/descriptor
    per cpu pair) is the bottleneck, so it is parallelized 4x across the 4
    SWDGE queues (each queue's descriptors are generated by its own Q7 cpu
    pair) and minimized: edges are sorted by (range-group, src) and equal-src
    runs are decomposed into K-edge units (K in {8,4,2,1}); one 256B src
    descriptor serves K edges (hu broadcast via step-0 AP). dst side stays
    one 256B descriptor per edge. Every gather is split into 128-aligned
    pieces of <=2048 descriptors spread greedily over the queues, so the
    per-queue serial quantum stays <=16us and the 4 Q7 pairs stay packed;
    8192-edge chunks keep the instruction count (and its ~0.6us/instruction
    fixed cost) low.
  - int16 gather indices => 4-way range bucketing (src>=32768, dst>=32768)
    with per-range base pointers; host permutes edges, unpermutes results.
  - DVE: hu broadcast across K members via step-0 AP, in-place multiply
    into the hv tile, segment-reduce 64-feature dim to one score per edge.
  - Output [128, tiles] stored contiguously; host transposes + scatters.
"""

import os
from contextlib import ExitStack

import numpy as np

import concourse.bacc as bacc
import concourse.mybir as mybir
from concourse import library_config
from concourse.bass import AP
from concourse._compat import get_trn_type
from concourse.bass_utils import run_bass_kernel_spmd

N_NODES = 50000
NPAD = 50008  # h padded so reads past the last node stay in bounds
D = 64
P = 128
N_CORES = 8
SPLIT = 32768
NQ = 4  # SWDGE queues (each with its own Q7 descriptor-gen cpu pair)
NB = 5  # buffer pairs

G_MAP = {8: 1024, 4: 2048, 2: 4096, 1: 4096}  # units per chunk

TRACE = False
LAST_RESULT = None


def _ensure_ntff_hook():
    """bass_utils' trace path imports antenv.axon_hooks, which this image's
    antenv package lacks. Recreate it from the boot helper so trace=True
    works; harmless no-op if the real module exists."""
    import sys
    import types

    try:
        import antenv.axon_hooks  # noqa: F401

        return
    except ImportError:
        pass
    try:
        import antenv
        from trn_agent_boot.trn_boot import _ntff_profile_via_ctypes

        hook = _ntff_profile_via_ctypes("/opt/axon/libaxon_pjrt.so")
        m = types.ModuleType("antenv.axon_hooks")
        m.get_axon_ntff_profile_hook = lambda: hook
        m.set_axon_ntff_profile_hook = lambda h: None
        sys.modules["antenv.axon_hooks"] = m
        antenv.axon_hooks = m
    except Exception:
        pass


def _wrap_idx(vals):
    """int16 index array [Npc] -> the [128, Npc/16] SBUF layout dma_gather
    expects (idx i at partition i%16, column i//16, replicated over the 8
    groups of 16 partitions — each SWDGE queue's Q7 pair reads its own
    group)."""
    w = vals.reshape(-1, 16).T  # [16, Npc/16]
    return np.ascontiguousarray(np.tile(w, (8, 1)))  # [128, Npc/16]


def _host_prep(src, dst):
    """Sort by (range-group, src); decompose equal-src runs into K-units.

    Returns (schedule, seqs, sidx_per_core, didx_per_core, u_total, e_total):
      schedule: list of (K, s_hi, d_hi, u_off, e_off, n_units), same all cores
      seqs: [N_CORES, e_total] global edge id per output position (-1 pad)
    """
    E = src.shape[0]
    g = (src >= SPLIT).astype(np.int8) * 2 + (dst >= SPLIT).astype(np.int8)
    order0 = np.lexsort((src, g))
    sg, ss, sd = g[order0], src[order0], dst[order0]

    new = np.ones(E, bool)
    new[1:] = (sg[1:] != sg[:-1]) | (ss[1:] != ss[:-1])
    run_start = np.flatnonzero(new)
    d = np.diff(np.append(run_start, E))
    run_id = np.cumsum(new) - 1
    r = np.arange(E) - run_start[run_id]
    dd = d[run_id]
    n8 = (dd // 8) * 8
    n4 = n8 + (((dd - n8) // 4) * 4)
    n2 = n4 + (((dd - n4) // 2) * 2)
    K_e = np.where(r < n8, 8, np.where(r < n4, 4, np.where(r < n2, 2, 1)))
    m_e = np.where(
        K_e == 8, r % 8,
        np.where(K_e == 4, (r - n8) % 4, np.where(K_e == 2, (r - n4) % 2, 0)),
    )
    first = m_e == 0

    pad_units = N_CORES * P
    schedule = []
    sidx_parts = [[] for _ in range(N_CORES)]
    didx_parts = [[] for _ in range(N_CORES)]
    seq_parts = [[] for _ in range(N_CORES)]
    u_off = 0
    e_off = 0
    for K in (8, 4, 2, 1):
        for gg in range(4):
            starts = np.flatnonzero(first & (K_e == K) & (sg == gg))
            if starts.size == 0:
                continue
            Upad = -(-starts.size // pad_units) * pad_units
            buf = np.full(Upad, -1, dtype=np.int64)
            buf[: starts.size] = starts
            U = Upad // N_CORES  # per-core units, multiple of 128
            s_hi, d_hi = gg >= 2, gg % 2 == 1
            for c in range(N_CORES):
                uc = buf[c * U : (c + 1) * U]
                valid = uc >= 0
                sv = np.zeros(U, np.int64)
                sv[valid] = ss[uc[valid]] - (SPLIT if s_hi else 0)
                sidx_parts[c].append(sv.astype(np.int16))
                dvals = np.zeros(U * K, np.int64)
                ids = np.full(U * K, -1, np.int64)
                uu = np.arange(U)
                for m in range(K):
                    pos = (K * (uu // P) + m) * P + uu % P
                    dvals[pos[valid]] = sd[uc[valid] + m] - (
                        SPLIT if d_hi else 0
                    )
                    ids[pos[valid]] = order0[uc[valid] + m]
                didx_parts[c].append(dvals.astype(np.int16))
                seq_parts[c].append(ids)
            # chunks
            o, rem = 0, U
            Gn = G_MAP[K]
            while rem > 0:
                n = min(Gn, rem)
                schedule.append((K, s_hi, d_hi, u_off + o, e_off + o * K, n))
                o += n
                rem -= n
            u_off += U
            e_off += U * K

    seqs = np.stack([np.concatenate(p) for p in seq_parts])
    sidx = [np.concatenate(p) for p in sidx_parts]
    didx = [np.concatenate(p) for p in didx_parts]
    return schedule, seqs, sidx, didx, u_off, e_off


def _build_nc(schedule, u_total, e_total):
    SCOLS = u_total // 16
    DCOLS = e_total // 16
    TILES = e_total // P

    nc = bacc.Bacc(
        get_trn_type() or "TRN2",
        debug=False,
        dynamic_dma_scratch_size=32768,
        num_swdge_queues=NQ,
    )
    h = nc.dram_tensor("h", [NPAD, D], mybir.dt.float32, kind="ExternalInput")
    sidx = nc.dram_tensor("sidx", [P, SCOLS], mybir.dt.int16, kind="ExternalInput")
    didx = nc.dram_tensor("didx", [P, DCOLS], mybir.dt.int16, kind="ExternalInput")
    out = nc.dram_tensor("out", [P, TILES], mybir.dt.float32, kind="ExternalOutput")

    # per-row base pointers for the two int16 index ranges
    h_lo = h[0:SPLIT, :]
    h_hi = h[SPLIT:NPAD, :]
    nch = len(schedule)

    # split each gather into 128-aligned pieces of <=2048 descriptors
    # (quanta small enough to pack the 4 Q7 pairs; pieces below 512 are
    # pathologically slow, so remainders fold into the last piece)
    def pieces(size):
        if size >= 4096:
            out = []
            rem = size
            while rem > 2048 + 1024:
                out.append(2048)
                rem -= 2048
            if rem > 2048:
                h = (rem // 2 // P) * P
                out += [h, rem - h]
            else:
                out.append(rem)
            return out
        if size >= 1024 and (size // 2) % P == 0:
            return [size // 2, size - size // 2]
        return [size]

    # greedy queue assignment balancing descriptor counts per piece, then an
    # offline move/swap pass to equalize per-queue totals (the online greedy
    # leaves a ~900-descriptor spread = ~7us of straggler time)
    plist = []  # [chunk, is_dst, q, sz]
    qloads = [0] * NQ
    for c, (K, s_hi, d_hi, uo, eo, n) in enumerate(schedule):
        for is_dst, sizes in ((0, pieces(n)), (1, pieces(n * K))):
            for sz in sizes:
                q = min(range(NQ), key=lambda x: qloads[x])
                qloads[q] += sz
                plist.append([c, is_dst, q, sz])
    for _ in range(400):
        mx = max(range(NQ), key=lambda q: qloads[q])
        mn = min(range(NQ), key=lambda q: qloads[q])
        gap = qloads[mx] - qloads[mn]
        if gap <= 128:
            break
        best, bred = None, 0
        for p in plist:  # move a piece mx -> mn
            if p[2] == mx and 0 < p[3] < gap and min(p[3], gap - p[3]) > bred:
                best, bred = (p, None), min(p[3], gap - p[3])
        for pa in plist:  # swap pieces between mx and mn
            if pa[2] != mx:
                continue
            for pb in plist:
                if pb[2] == mn and 0 < pa[3] - pb[3] < gap:
                    d = pa[3] - pb[3]
                    if min(d, gap - d) > bred:
                        best, bred = (pa, pb), min(d, gap - d)
        if best is None:
            break
        pa, pb = best
        qloads[mx] -= pa[3]
        qloads[mn] += pa[3]
        pa[2] = mn
        if pb is not None:
            qloads[mn] -= pb[3]
            qloads[mx] += pb[3]
            pb[2] = mx
    qassign = [([], []) for _ in schedule]
    for c, is_dst, q, sz in plist:
        qassign[c][is_dst].append((q, sz))

    with ExitStack() as stack:
        ent = stack.enter_context
        hu = [ent(nc.sbuf_tensor(f"hu{i}", [P, 2048], mybir.dt.float32)) for i in range(NB)]
        hv = [ent(nc.sbuf_tensor(f"hv{i}", [P, 4096], mybir.dt.float32)) for i in range(NB)]
        sidx_sb = ent(nc.sbuf_tensor("sidx_sb", [P, SCOLS], mybir.dt.int16))
        didx_sb = ent(nc.sbuf_tensor("didx_sb", [P, DCOLS], mybir.dt.int16))
        outb = ent(nc.sbuf_tensor("outb", [P, TILES], mybir.dt.float32))
        io = ent(nc.semaphore("io"))
        io2 = ent(nc.semaphore("io2"))
        gsem = [ent(nc.semaphore(f"g{i}")) for i in range(NB)]
        vsem = [ent(nc.semaphore(f"v{i}")) for i in range(NB)]
        mr = ent(nc.semaphore("mr"))

        def hu_ap(b, t_u, off=0):
            base = hu[b][:]
            return AP(base.tensor, off * D, [[2048, P], [D, t_u], [1, D]])

        def hu_bcast(b, t_u, K):
            base = hu[b][:]
            return AP(base.tensor, 0, [[2048, P], [D, t_u], [0, K], [1, D]])

        def hv_ap(b, t_e, off=0):
            base = hv[b][:]
            return AP(base.tensor, off * D, [[4096, P], [D, t_e], [1, D]])

        def hv_4d(b, t_u, K):
            base = hv[b][:]
            return AP(base.tensor, 0, [[4096, P], [D * K, t_u], [D, K], [1, D]])

        with nc.Block() as block:

            @block.sync
            def _(sync):
                sync.dma_start(sidx_sb[:], sidx[:]).then_inc(io, 16)
                sync.dma_start(didx_sb[:], didx[:]).then_inc(io, 16)
                for b in range(NB):
                    uses = (nch - b + NB - 1) // NB
                    if uses:
                        sync.wait_ge(vsem[b], uses)
                sync.dma_start(out[:], outb[:]).then_inc(io2, 16)
                sync.wait_ge(io2, 16)

            @block.gpsimd
            def _(gp):
                gp.load_library(library_config.mlp)
                gp.wait_ge(io, 32)
                for c, (K, s_hi, d_hi, uo, eo, n) in enumerate(schedule):
                    b = c % NB
                    sps, dps = qassign[c]
                    if c >= NB:
                        gp.wait_ge(vsem[b], c // NB)
                    off = 0
                    for q, sz in sps:
                        gp.dma_gather(
                            hu_ap(b, sz // P, off=off // P),
                            h_hi if s_hi else h_lo,
                            sidx_sb[:, (uo + off) // 16 : (uo + off + sz) // 16],
                            sz,
                            sz,
                            D,
                            single_packet=False,
                            queue_num=q,
                        ).then_inc(gsem[b], 16)
                        off += sz
                    off = 0
                    for q, sz in dps:
                        gp.dma_gather(
                            hv_ap(b, sz // P, off=off // P),
                            h_hi if d_hi else h_lo,
                            didx_sb[:, (eo + off) // 16 : (eo + off + sz) // 16],
                            sz,
                            sz,
                            D,
                            single_packet=False,
                            queue_num=q,
                        ).then_inc(gsem[b], 16)
                        off += sz

            @block.vector
            def _(ve):
                gs_acc = {}
                gs_needed = []
                for c in range(nch):
                    bb = c % NB
                    sps, dps = qassign[c]
                    gs_acc[bb] = gs_acc.get(bb, 0) + 16 * (len(sps) + len(dps))
                    gs_needed.append(gs_acc[bb])
                for c, (K, s_hi, d_hi, uo, eo, n) in enumerate(schedule):
                    b = c % NB
                    ve.wait_ge(gsem[b], gs_needed[c])
                    t_u = n // P
                    t_e = t_u * K
                    if K == 1:
                        prod_in1 = hu_ap(b, t_u)
                        prod = hv_ap(b, t_e)
                    else:
                        prod_in1 = hu_bcast(b, t_u, K)
                        prod = hv_4d(b, t_u, K)
                    ve.tensor_tensor(
                        out=prod, in0=prod, in1=prod_in1,
                        op=mybir.AluOpType.mult,
                    ).then_inc(mr, 1)
                    ve.wait_ge(mr, c + 1)
                    ve.tensor_reduce(
                        out=outb[:, eo // P : eo // P + t_e],
                        in_=prod,
                        axis=mybir.AxisListType.X,
                        op=mybir.AluOpType.add,
                    ).then_inc(vsem[b], 1)

    nc.compile()
    return nc


def kernel(h, src, dst):
    global LAST_RESULT
    h = np.asarray(h, dtype=np.float32)
    hp = np.zeros((NPAD, D), np.float32)
    hp[:N_NODES] = h
    src = np.asarray(src).astype(np.int64)
    dst = np.asarray(dst).astype(np.int64)
    E = src.shape[0]

    schedule, seqs, sidx, didx, u_total, e_total = _host_prep(src, dst)
    in_maps = [
        {"h": hp, "sidx": _wrap_idx(sidx[c]), "didx": _wrap_idx(didx[c])}
        for c in range(N_CORES)
    ]
    nc = _build_nc(schedule, u_total, e_total)

    if TRACE or os.environ.get("BASS_TRACE"):
        _ensure_ntff_hook()
    res = run_bass_kernel_spmd(nc, in_maps, core_ids=list(range(N_CORES)), trace=TRACE)
    LAST_RESULT = res

    out = np.empty(E, np.float32)
    for c in range(N_CORES):
        dots = res.results[c]["out"].T.reshape(-1)
        seq = seqs[c]
        valid = seq >= 0
        out[seq[valid]] = dots[valid]
    return out


# revision 50
# speedup vs baseline: 1.0570x; 1.0570x over previous
"""Trainium2 Bass kernel for per-edge dot products (GNN DotPredictor).

out[e] = sum(h[src[e]] * h[dst[e]]); 800k edges, h [50k, 64] f32, 8 cores.

Design (v7):
  - Edges sharded 8 ways; h replicated. Per-edge rows fetched from HBM with
    the Q7 `dma_gather` path. The Q7 descriptor generation (~8ns/descriptor
    per cpu pair) is the bottleneck, so it is parallelized 4x across the 4
    SWDGE queues (each queue's descriptors are generated by its own Q7 cpu
    pair) and minimized: edges are sorted by (range-group, src) and equal-src
    runs are decomposed into K-edge units (K in {8,4,2,1}); one 256B src
    descriptor serves K edges (hu broadcast via step-0 AP). dst side stays
    one 256B descriptor per edge. Every gather is split into 128-aligned
    pieces of <=2048 descriptors spread greedily over the queues, so the
    per-queue serial quantum stays <=16us and the 4 Q7 pairs stay packed;
    8192-edge chunks keep the instruction count (and its ~0.6us/instruction
    fixed cost) low.
  - int16 gather indices => 4-way range bucketing (src>=32768, dst>=32768)
    with per-range base pointers; host permutes edges, unpermutes results.
  - DVE: hu broadcast across K members via step-0 AP, in-place multiply
    into the hv tile, segment-reduce 64-feature dim to one score per edge.
  - Output [128, tiles] stored contiguously; host transposes + scatters.
"""

import os
from contextlib import ExitStack

import numpy as np

import concourse.bacc as bacc
import concourse.mybir as mybir
from concourse import library_config
from concourse.bass import AP
from concourse._compat import get_trn_type
from concourse.bass_utils import run_bass_kernel_spmd

N_NODES = 50000
NPAD = 50008  # h padded so reads past the last node stay in bounds
D = 64
P = 128
N_CORES = 8
SPLIT = 32768
NQ = 4  # SWDGE queues (each with its own Q7 descriptor-gen cpu pair)
NB = 5  # buffer pairs

G_MAP = {8: 1024, 4: 2048, 2: 4096, 1: 4096}  # units per chunk

TRACE = False
LAST_RESULT = None


def _ensure_ntff_hook():
    """bass_utils' trace path imports antenv.axon_hooks, which this image's
    antenv package lacks. Recreate it from the boot helper so trace=True
    works; harmless no-op if the real module exists."""
    import sys
    import types

    try:
        import antenv.axon_hooks  # noqa: F401

        return
    except ImportError:
        pass
    try:
        import antenv
        from trn_agent_boot.trn_boot import _ntff_profile_via_ctypes

        hook = _ntff_profile_via_ctypes("/opt/axon/libaxon_pjrt.so")
        m = types.ModuleType("antenv.axon_hooks")
        m.get_axon_ntff_profile_hook = lambda: hook
        m.set_axon_ntff_profile_hook = lambda h: None
        sys.modules["antenv.axon_hooks"] = m
        antenv.axon_hooks = m
    except Exception:
        pass


def _wrap_idx(vals):
    """int16 index array [Npc] -> the [128, Npc/16] SBUF layout dma_gather
    expects (idx i at partition i%16, column i//16, replicated over the 8
    groups of 16 partitions — each SWDGE queue's Q7 pair reads its own
    group)."""
    w = vals.reshape(-1, 16).T  # [16, Npc/16]
    return np.ascontiguousarray(np.tile(w, (8, 1)))  # [128, Npc/16]


def _host_prep(src, dst):
    """Sort by (range-group, src); decompose equal-src runs into K-units.

    Returns (schedule, seqs, sidx_per_core, didx_per_core, u_total, e_total):
      schedule: list of (K, s_hi, d_hi, u_off, e_off, n_units), same all cores
      seqs: [N_CORES, e_total] global edge id per output position (-1 pad)
    """
    E = src.shape[0]
    g = (src >= SPLIT).astype(np.int8) * 2 + (dst >= SPLIT).astype(np.int8)
    order0 = np.lexsort((src, g))
    sg, ss, sd = g[order0], src[order0], dst[order0]

    new = np.ones(E, bool)
    new[1:] = (sg[1:] != sg[:-1]) | (ss[1:] != ss[:-1])
    run_start = np.flatnonzero(new)
    d = np.diff(np.append(run_start, E))
    run_id = np.cumsum(new) - 1
    r = np.arange(E) - run_start[run_id]
    dd = d[run_id]
    n8 = (dd // 8) * 8
    n4 = n8 + (((dd - n8) // 4) * 4)
    n2 = n4 + (((dd - n4) // 2) * 2)
    K_e = np.where(r < n8, 8, np.where(r < n4, 4, np.where(r < n2, 2, 1)))
    m_e = np.where(
        K_e == 8, r % 8,
        np.where(K_e == 4, (r - n8) % 4, np.where(K_e == 2, (r - n4) % 2, 0)),
    )
    first = m_e == 0

    pad_units = N_CORES * P
    schedule = []
    sidx_parts = [[] for _ in range(N_CORES)]
    didx_parts = [[] for _ in range(N_CORES)]
    seq_parts = [[] for _ in range(N_CORES)]
    u_off = 0
    e_off = 0
    for K in (8, 4, 2, 1):
        for gg in range(4):
            starts = np.flatnonzero(first & (K_e == K) & (sg == gg))
            if starts.size == 0:
                continue
            Upad = -(-starts.size // pad_units) * pad_units
            buf = np.full(Upad, -1, dtype=np.int64)
            buf[: starts.size] = starts
            U = Upad // N_CORES  # per-core units, multiple of 128
            s_hi, d_hi = gg >= 2, gg % 2 == 1
            for c in range(N_CORES):
                uc = buf[c * U : (c + 1) * U]
                valid = uc >= 0
                sv = np.zeros(U, np.int64)
                sv[valid] = ss[uc[valid]] - (SPLIT if s_hi else 0)
                sidx_parts[c].append(sv.astype(np.int16))
                dvals = np.zeros(U * K, np.int64)
                ids = np.full(U * K, -1, np.int64)
                uu = np.arange(U)
                for m in range(K):
                    pos = (K * (uu // P) + m) * P + uu % P
                    dvals[pos[valid]] = sd[uc[valid] + m] - (
                        SPLIT if d_hi else 0
                    )
                    ids[pos[valid]] = order0[uc[valid] + m]
                didx_parts[c].append(dvals.astype(np.int16))
                seq_parts[c].append(ids)
            # chunks
            o, rem = 0, U
            Gn = G_MAP[K]
            while rem > 0:
                n = min(Gn, rem)
                schedule.append((K, s_hi, d_hi, u_off + o, e_off + o * K, n))
                o += n
                rem -= n
            u_off += U
            e_off += U * K

    seqs = np.stack([np.concatenate(p) for p in seq_parts])
    sidx = [np.concatenate(p) for p in sidx_parts]
    didx = [np.concatenate(p) for p in didx_parts]
    return schedule, seqs, sidx, didx, u_off, e_off


def _build_nc(schedule, u_total, e_total):
    SCOLS = u_total // 16
    DCOLS = e_total // 16
    TILES = e_total // P

    nc = bacc.Bacc(
        get_trn_type() or "TRN2",
        debug=False,
        dynamic_dma_scratch_size=32768,
        num_swdge_queues=NQ,
    )
    h = nc.dram_tensor("h", [NPAD, D], mybir.dt.float32, kind="ExternalInput")
    sidx = nc.dram_tensor("sidx", [P, SCOLS], mybir.dt.int16, kind="ExternalInput")
    didx = nc.dram_tensor("didx", [P, DCOLS], mybir.dt.int16, kind="ExternalInput")
    out = nc.dram_tensor("out", [P, TILES], mybir.dt.float32, kind="ExternalOutput")

    # per-row base pointers for the two int16 index ranges
    h_lo = h[0:SPLIT, :]
    h_hi = h[SPLIT:NPAD, :]
    nch = len(schedule)

    # split each gather into 128-aligned pieces of <=2048 descriptors
    # (quanta small enough to pack the 4 Q7 pairs; pieces below 512 are
    # pathologically slow, so remainders fold into the last piece)
    def pieces(size):
        if size >= 4096:
            out = []
            rem = size
            while rem > 2048 + 1024:
                out.append(2048)
                rem -= 2048
            if rem > 2048:
                h = (rem // 2 // P) * P
                out += [h, rem - h]
            else:
                out.append(rem)
            return out
        if size >= 1024 and (size // 2) % P == 0:
            return [size // 2, size - size // 2]
        return [size]

    # greedy queue assignment balancing descriptor counts per piece
    qloads = [0] * NQ
    qassign = []
    for (K, s_hi, d_hi, uo, eo, n) in schedule:
        sps, dps = [], []
        for sz in pieces(n):
            q = min(range(NQ), key=lambda x: qloads[x])
            qloads[q] += sz
            sps.append((q, sz))
        for sz in pieces(n * K):
            q = min(range(NQ), key=lambda x: qloads[x])
            qloads[q] += sz
            dps.append((q, sz))
        qassign.append((sps, dps))

    with ExitStack() as stack:
        ent = stack.enter_context
        hu = [ent(nc.sbuf_tensor(f"hu{i}", [P, 2048], mybir.dt.float32)) for i in range(NB)]
        hv = [ent(nc.sbuf_tensor(f"hv{i}", [P, 4096], mybir.dt.float32)) for i in range(NB)]
        sidx_sb = ent(nc.sbuf_tensor("sidx_sb", [P, SCOLS], mybir.dt.int16))
        didx_sb = ent(nc.sbuf_tensor("didx_sb", [P, DCOLS], mybir.dt.int16))
        outb = ent(nc.sbuf_tensor("outb", [P, TILES], mybir.dt.float32))
        io = ent(nc.semaphore("io"))
        io2 = ent(nc.semaphore("io2"))
        gsem = [ent(nc.semaphore(f"g{i}")) for i in range(NB)]
        vsem = [ent(nc.semaphore(f"v{i}")) for i in range(NB)]
        mr = ent(nc.semaphore("mr"))

        def hu_ap(b, t_u, off=0):
            base = hu[b][:]
            return AP(base.tensor, off * D, [[2048, P], [D, t_u], [1, D]])

        def hu_bcast(b, t_u, K):
            base = hu[b][:]
            return AP(base.tensor, 0, [[2048, P], [D, t_u], [0, K], [1, D]])

        def hv_ap(b, t_e, off=0):
            base = hv[b][:]
            return AP(base.tensor, off * D, [[4096, P], [D, t_e], [1, D]])

        def hv_4d(b, t_u, K):
            base = hv[b][:]
            return AP(base.tensor, 0, [[4096, P], [D * K, t_u], [D, K], [1, D]])

        with nc.Block() as block:

            @block.sync
            def _(sync):
                sync.dma_start(sidx_sb[:], sidx[:]).then_inc(io, 16)
                sync.dma_start(didx_sb[:], didx[:]).then_inc(io, 16)
                # store the output round by round so only the last ~1/5 of it
                # remains after the final reduce
                nrounds = -(-nch // NB)
                for r in range(nrounds):
                    c_lo, c_hi = r * NB, min(nch, (r + 1) * NB)
                    for c in range(c_lo, c_hi):
                        sync.wait_ge(vsem[c % NB], c // NB + 1)
                    eo_lo = schedule[c_lo][4]
                    eo_hi = schedule[c_hi][4] if c_hi < nch else e_total
                    sync.dma_start(
                        out[:, eo_lo // P : eo_hi // P],
                        outb[:, eo_lo // P : eo_hi // P],
                    ).then_inc(io2, 16)
                sync.wait_ge(io2, 16 * nrounds)

            @block.gpsimd
            def _(gp):
                gp.load_library(library_config.mlp)
                gp.wait_ge(io, 32)
                for c, (K, s_hi, d_hi, uo, eo, n) in enumerate(schedule):
                    b = c % NB
                    sps, dps = qassign[c]
                    if c >= NB:
                        gp.wait_ge(vsem[b], c // NB)
                    off = 0
                    for q, sz in sps:
                        gp.dma_gather(
                            hu_ap(b, sz // P, off=off // P),
                            h_hi if s_hi else h_lo,
                            sidx_sb[:, (uo + off) // 16 : (uo + off + sz) // 16],
                            sz,
                            sz,
                            D,
                            single_packet=False,
                            queue_num=q,
                        ).then_inc(gsem[b], 16)
                        off += sz
                    off = 0
                    for q, sz in dps:
                        gp.dma_gather(
                            hv_ap(b, sz // P, off=off // P),
                            h_hi if d_hi else h_lo,
                            didx_sb[:, (eo + off) // 16 : (eo + off + sz) // 16],
                            sz,
                            sz,
                            D,
                            single_packet=False,
                            queue_num=q,
                        ).then_inc(gsem[b], 16)
                        off += sz

            @block.vector
            def _(ve):
                gs_acc = {}
                gs_needed = []
                for c in range(nch):
                    bb = c % NB
                    sps, dps = qassign[c]
                    gs_acc[bb] = gs_acc.get(bb, 0) + 16 * (len(sps) + len(dps))
                    gs_needed.append(gs_acc[bb])
                for c, (K, s_hi, d_hi, uo, eo, n) in enumerate(schedule):
                    b = c % NB
                    ve.wait_ge(gsem[b], gs_needed[c])
                    t_u = n // P
                    t_e = t_u * K
                    if K == 1:
                        prod_in1 = hu_ap(b, t_u)
                        prod = hv_ap(b, t_e)
                    else:
                        prod_in1 = hu_bcast(b, t_u, K)
                        prod = hv_4d(b, t_u, K)
                    ve.tensor_tensor(
                        out=prod, in0=prod, in1=prod_in1,
                        op=mybir.AluOpType.mult,
                    ).then_inc(mr, 1)
                    ve.wait_ge(mr, c + 1)
                    ve.tensor_reduce(
                        out=outb[:, eo // P : eo // P + t_e],
                        in_=prod,
                        axis=mybir.AxisListType.X,
                        op=mybir.AluOpType.add,
                    ).then_inc(vsem[b], 1)

    nc.compile()
    return nc


def kernel(h, src, dst):
    global LAST_RESULT
    h = np.asarray(h, dtype=np.float32)
    hp = np.zeros((NPAD, D), np.float32)
    hp[:N_NODES] = h
    src = np.asarray(src).astype(np.int64)
    dst = np.asarray(dst).astype(np.int64)
    E = src.shape[0]

    schedule, seqs, sidx, didx, u_total, e_total = _host_prep(src, dst)
    in_maps = [
        {"h": hp, "sidx": _wrap_idx(sidx[c]), "didx": _wrap_idx(didx[c])}
        for c in range(N_CORES)
    ]
    nc = _build_nc(schedule, u_total, e_total)

    if TRACE or os.environ.get("BASS_TRACE"):
        _ensure_ntff_hook()
    res = run_bass_kernel_spmd(nc, in_maps, core_ids=list(range(N_CORES)), trace=TRACE)
    LAST_RESULT = res

    out = np.empty(E, np.float32)
    for c in range(N_CORES):
        dots = res.results[c]["out"].T.reshape(-1)
        seq = seqs[c]
        valid = seq >= 0
        out[seq[valid]] = dots[valid]
    return out


# revision 51
# speedup vs baseline: 1.0617x; 1.0044x over previous
"""Trainium2 Bass kernel for per-edge dot products (GNN DotPredictor).

out[e] = sum(h[src[e]] * h[dst[e]]); 800k edges, h [50k, 64] f32, 8 cores.

Design (v7):
  - Edges sharded 8 ways; h replicated. Per-edge rows fetched from HBM with
    the Q7 `dma_gather` path. The Q7 descriptor generation (~8ns/descriptor
    per cpu pair) is the bottleneck, so it is parallelized 4x across the 4
    SWDGE queues (each queue's descriptors are generated by its own Q7 cpu
    pair) and minimized: edges are sorted by (range-group, src) and equal-src
    runs are decomposed into K-edge units (K in {8,4,2,1}); one 256B src
    descriptor serves K edges (hu broadcast via step-0 AP). dst side stays
    one 256B descriptor per edge. Every gather is split into 128-aligned
    pieces of <=2048 descriptors spread greedily over the queues, so the
    per-queue serial quantum stays <=16us and the 4 Q7 pairs stay packed;
    8192-edge chunks keep the instruction count (and its ~0.6us/instruction
    fixed cost) low.
  - int16 gather indices => 4-way range bucketing (src>=32768, dst>=32768)
    with per-range base pointers; host permutes edges, unpermutes results.
  - DVE: hu broadcast across K members via step-0 AP, in-place multiply
    into the hv tile, segment-reduce 64-feature dim to one score per edge.
  - Output [128, tiles] stored contiguously; host transposes + scatters.
"""

import os
from contextlib import ExitStack

import numpy as np

import concourse.bacc as bacc
import concourse.mybir as mybir
from concourse import library_config
from concourse.bass import AP
from concourse._compat import get_trn_type
from concourse.bass_utils import run_bass_kernel_spmd

N_NODES = 50000
NPAD = 50008  # h padded so reads past the last node stay in bounds
D = 64
P = 128
N_CORES = 8
SPLIT = 32768
NQ = 4  # SWDGE queues (each with its own Q7 descriptor-gen cpu pair)
NB = 5  # buffer pairs

G_MAP = {8: 1024, 4: 2048, 2: 4096, 1: 2048}  # units per chunk (small K=1 chunks end the schedule with a short DVE tail)

TRACE = False
LAST_RESULT = None


def _ensure_ntff_hook():
    """bass_utils' trace path imports antenv.axon_hooks, which this image's
    antenv package lacks. Recreate it from the boot helper so trace=True
    works; harmless no-op if the real module exists."""
    import sys
    import types

    try:
        import antenv.axon_hooks  # noqa: F401

        return
    except ImportError:
        pass
    try:
        import antenv
        from trn_agent_boot.trn_boot import _ntff_profile_via_ctypes

        hook = _ntff_profile_via_ctypes("/opt/axon/libaxon_pjrt.so")
        m = types.ModuleType("antenv.axon_hooks")
        m.get_axon_ntff_profile_hook = lambda: hook
        m.set_axon_ntff_profile_hook = lambda h: None
        sys.modules["antenv.axon_hooks"] = m
        antenv.axon_hooks = m
    except Exception:
        pass


def _wrap_idx(vals):
    """int16 index array [Npc] -> the [128, Npc/16] SBUF layout dma_gather
    expects (idx i at partition i%16, column i//16, replicated over the 8
    groups of 16 partitions — each SWDGE queue's Q7 pair reads its own
    group)."""
    w = vals.reshape(-1, 16).T  # [16, Npc/16]
    return np.ascontiguousarray(np.tile(w, (8, 1)))  # [128, Npc/16]


def _host_prep(src, dst):
    """Sort by (range-group, src); decompose equal-src runs into K-units.

    Returns (schedule, seqs, sidx_per_core, didx_per_core, u_total, e_total):
      schedule: list of (K, s_hi, d_hi, u_off, e_off, n_units), same all cores
      seqs: [N_CORES, e_total] global edge id per output position (-1 pad)
    """
    E = src.shape[0]
    g = (src >= SPLIT).astype(np.int8) * 2 + (dst >= SPLIT).astype(np.int8)
    order0 = np.lexsort((src, g))
    sg, ss, sd = g[order0], src[order0], dst[order0]

    new = np.ones(E, bool)
    new[1:] = (sg[1:] != sg[:-1]) | (ss[1:] != ss[:-1])
    run_start = np.flatnonzero(new)
    d = np.diff(np.append(run_start, E))
    run_id = np.cumsum(new) - 1
    r = np.arange(E) - run_start[run_id]
    dd = d[run_id]
    n8 = (dd // 8) * 8
    n4 = n8 + (((dd - n8) // 4) * 4)
    n2 = n4 + (((dd - n4) // 2) * 2)
    K_e = np.where(r < n8, 8, np.where(r < n4, 4, np.where(r < n2, 2, 1)))
    m_e = np.where(
        K_e == 8, r % 8,
        np.where(K_e == 4, (r - n8) % 4, np.where(K_e == 2, (r - n4) % 2, 0)),
    )
    first = m_e == 0

    pad_units = N_CORES * P
    schedule = []
    sidx_parts = [[] for _ in range(N_CORES)]
    didx_parts = [[] for _ in range(N_CORES)]
    seq_parts = [[] for _ in range(N_CORES)]
    u_off = 0
    e_off = 0
    for K in (8, 4, 2, 1):
        for gg in range(4):
            starts = np.flatnonzero(first & (K_e == K) & (sg == gg))
            if starts.size == 0:
                continue
            Upad = -(-starts.size // pad_units) * pad_units
            buf = np.full(Upad, -1, dtype=np.int64)
            buf[: starts.size] = starts
            U = Upad // N_CORES  # per-core units, multiple of 128
            s_hi, d_hi = gg >= 2, gg % 2 == 1
            for c in range(N_CORES):
                uc = buf[c * U : (c + 1) * U]
                valid = uc >= 0
                sv = np.zeros(U, np.int64)
                sv[valid] = ss[uc[valid]] - (SPLIT if s_hi else 0)
                sidx_parts[c].append(sv.astype(np.int16))
                dvals = np.zeros(U * K, np.int64)
                ids = np.full(U * K, -1, np.int64)
                uu = np.arange(U)
                for m in range(K):
                    pos = (K * (uu // P) + m) * P + uu % P
                    dvals[pos[valid]] = sd[uc[valid] + m] - (
                        SPLIT if d_hi else 0
                    )
                    ids[pos[valid]] = order0[uc[valid] + m]
                didx_parts[c].append(dvals.astype(np.int16))
                seq_parts[c].append(ids)
            # chunks
            o, rem = 0, U
            Gn = G_MAP[K]
            while rem > 0:
                n = min(Gn, rem)
                schedule.append((K, s_hi, d_hi, u_off + o, e_off + o * K, n))
                o += n
                rem -= n
            u_off += U
            e_off += U * K

    seqs = np.stack([np.concatenate(p) for p in seq_parts])
    sidx = [np.concatenate(p) for p in sidx_parts]
    didx = [np.concatenate(p) for p in didx_parts]
    return schedule, seqs, sidx, didx, u_off, e_off


def _build_nc(schedule, u_total, e_total):
    SCOLS = u_total // 16
    DCOLS = e_total // 16
    TILES = e_total // P

    nc = bacc.Bacc(
        get_trn_type() or "TRN2",
        debug=False,
        dynamic_dma_scratch_size=32768,
        num_swdge_queues=NQ,
    )
    h = nc.dram_tensor("h", [NPAD, D], mybir.dt.float32, kind="ExternalInput")
    sidx = nc.dram_tensor("sidx", [P, SCOLS], mybir.dt.int16, kind="ExternalInput")
    didx = nc.dram_tensor("didx", [P, DCOLS], mybir.dt.int16, kind="ExternalInput")
    out = nc.dram_tensor("out", [P, TILES], mybir.dt.float32, kind="ExternalOutput")

    # per-row base pointers for the two int16 index ranges
    h_lo = h[0:SPLIT, :]
    h_hi = h[SPLIT:NPAD, :]
    nch = len(schedule)

    # split each gather into 128-aligned pieces of <=2048 descriptors
    # (quanta small enough to pack the 4 Q7 pairs; pieces below 512 are
    # pathologically slow, so remainders fold into the last piece)
    def pieces(size):
        if size >= 4096:
            out = []
            rem = size
            while rem > 2048 + 1024:
                out.append(2048)
                rem -= 2048
            if rem > 2048:
                h = (rem // 2 // P) * P
                out += [h, rem - h]
            else:
                out.append(rem)
            return out
        if size >= 1024 and (size // 2) % P == 0:
            return [size // 2, size - size // 2]
        return [size]

    # greedy queue assignment balancing descriptor counts per piece
    qloads = [0] * NQ
    qassign = []
    for (K, s_hi, d_hi, uo, eo, n) in schedule:
        sps, dps = [], []
        for sz in pieces(n):
            q = min(range(NQ), key=lambda x: qloads[x])
            qloads[q] += sz
            sps.append((q, sz))
        for sz in pieces(n * K):
            q = min(range(NQ), key=lambda x: qloads[x])
            qloads[q] += sz
            dps.append((q, sz))
        qassign.append((sps, dps))

    with ExitStack() as stack:
        ent = stack.enter_context
        hu = [ent(nc.sbuf_tensor(f"hu{i}", [P, 2048], mybir.dt.float32)) for i in range(NB)]
        hv = [ent(nc.sbuf_tensor(f"hv{i}", [P, 4096], mybir.dt.float32)) for i in range(NB)]
        sidx_sb = ent(nc.sbuf_tensor("sidx_sb", [P, SCOLS], mybir.dt.int16))
        didx_sb = ent(nc.sbuf_tensor("didx_sb", [P, DCOLS], mybir.dt.int16))
        outb = ent(nc.sbuf_tensor("outb", [P, TILES], mybir.dt.float32))
        io = ent(nc.semaphore("io"))
        io2 = ent(nc.semaphore("io2"))
        gsem = [ent(nc.semaphore(f"g{i}")) for i in range(NB)]
        vsem = [ent(nc.semaphore(f"v{i}")) for i in range(NB)]
        mr = ent(nc.semaphore("mr"))

        def hu_ap(b, t_u, off=0):
            base = hu[b][:]
            return AP(base.tensor, off * D, [[2048, P], [D, t_u], [1, D]])

        def hu_bcast(b, t_u, K):
            base = hu[b][:]
            return AP(base.tensor, 0, [[2048, P], [D, t_u], [0, K], [1, D]])

        def hv_ap(b, t_e, off=0):
            base = hv[b][:]
            return AP(base.tensor, off * D, [[4096, P], [D, t_e], [1, D]])

        def hv_4d(b, t_u, K):
            base = hv[b][:]
            return AP(base.tensor, 0, [[4096, P], [D * K, t_u], [D, K], [1, D]])

        with nc.Block() as block:

            @block.sync
            def _(sync):
                sync.dma_start(sidx_sb[:], sidx[:]).then_inc(io, 16)
                sync.dma_start(didx_sb[:], didx[:]).then_inc(io, 16)
                # store the output round by round so only the last ~1/5 of it
                # remains after the final reduce
                nrounds = -(-nch // NB)
                for r in range(nrounds):
                    c_lo, c_hi = r * NB, min(nch, (r + 1) * NB)
                    for c in range(c_lo, c_hi):
                        sync.wait_ge(vsem[c % NB], c // NB + 1)
                    eo_lo = schedule[c_lo][4]
                    eo_hi = schedule[c_hi][4] if c_hi < nch else e_total
                    sync.dma_start(
                        out[:, eo_lo // P : eo_hi // P],
                        outb[:, eo_lo // P : eo_hi // P],
                    ).then_inc(io2, 16)
                sync.wait_ge(io2, 16 * nrounds)

            @block.gpsimd
            def _(gp):
                gp.load_library(library_config.mlp)
                gp.wait_ge(io, 32)
                for c, (K, s_hi, d_hi, uo, eo, n) in enumerate(schedule):
                    b = c % NB
                    sps, dps = qassign[c]
                    if c >= NB:
                        gp.wait_ge(vsem[b], c // NB)
                    off = 0
                    for q, sz in sps:
                        gp.dma_gather(
                            hu_ap(b, sz // P, off=off // P),
                            h_hi if s_hi else h_lo,
                            sidx_sb[:, (uo + off) // 16 : (uo + off + sz) // 16],
                            sz,
                            sz,
                            D,
                            single_packet=False,
                            queue_num=q,
                        ).then_inc(gsem[b], 16)
                        off += sz
                    off = 0
                    for q, sz in dps:
                        gp.dma_gather(
                            hv_ap(b, sz // P, off=off // P),
                            h_hi if d_hi else h_lo,
                            didx_sb[:, (eo + off) // 16 : (eo + off + sz) // 16],
                            sz,
                            sz,
                            D,
                            single_packet=False,
                            queue_num=q,
                        ).then_inc(gsem[b], 16)
                        off += sz

            @block.vector
            def _(ve):
                gs_acc = {}
                gs_needed = []
                for c in range(nch):
                    bb = c % NB
                    sps, dps = qassign[c]
                    gs_acc[bb] = gs_acc.get(bb, 0) + 16 * (len(sps) + len(dps))
                    gs_needed.append(gs_acc[bb])
                for c, (K, s_hi, d_hi, uo, eo, n) in enumerate(schedule):
                    b = c % NB
                    ve.wait_ge(gsem[b], gs_needed[c])
                    t_u = n // P
                    t_e = t_u * K
                    if K == 1:
                        prod_in1 = hu_ap(b, t_u)
                        prod = hv_ap(b, t_e)
                    else:
                        prod_in1 = hu_bcast(b, t_u, K)
                        prod = hv_4d(b, t_u, K)
                    ve.tensor_tensor(
                        out=prod, in0=prod, in1=prod_in1,
                        op=mybir.AluOpType.mult,
                    ).then_inc(mr, 1)
                    ve.wait_ge(mr, c + 1)
                    ve.tensor_reduce(
                        out=outb[:, eo // P : eo // P + t_e],
                        in_=prod,
                        axis=mybir.AxisListType.X,
                        op=mybir.AluOpType.add,
                    ).then_inc(vsem[b], 1)

    nc.compile()
    return nc


def kernel(h, src, dst):
    global LAST_RESULT
    h = np.asarray(h, dtype=np.float32)
    hp = np.zeros((NPAD, D), np.float32)
    hp[:N_NODES] = h
    src = np.asarray(src).astype(np.int64)
    dst = np.asarray(dst).astype(np.int64)
    E = src.shape[0]

    schedule, seqs, sidx, didx, u_total, e_total = _host_prep(src, dst)
    in_maps = [
        {"h": hp, "sidx": _wrap_idx(sidx[c]), "didx": _wrap_idx(didx[c])}
        for c in range(N_CORES)
    ]
    nc = _build_nc(schedule, u_total, e_total)

    if TRACE or os.environ.get("BASS_TRACE"):
        _ensure_ntff_hook()
    res = run_bass_kernel_spmd(nc, in_maps, core_ids=list(range(N_CORES)), trace=TRACE)
    LAST_RESULT = res

    out = np.empty(E, np.float32)
    for c in range(N_CORES):
        dots = res.results[c]["out"].T.reshape(-1)
        seq = seqs[c]
        valid = seq >= 0
        out[seq[valid]] = dots[valid]
    return out


# revision 52
# speedup vs baseline: 1.0660x; 1.0041x over previous
"""Trainium2 Bass kernel for per-edge dot products (GNN DotPredictor).

out[e] = sum(h[src[e]] * h[dst[e]]); 800k edges, h [50k, 64] f32, 8 cores.

Design (v7):
  - Edges sharded 8 ways; h replicated. Per-edge rows fetched from HBM with
    the Q7 `dma_gather` path. The Q7 descriptor generation (~8ns/descriptor
    per cpu pair) is the bottleneck, so it is parallelized 4x across the 4
    SWDGE queues (each queue's descriptors are generated by its own Q7 cpu
    pair) and minimized: edges are sorted by (range-group, src) and equal-src
    runs are decomposed into K-edge units (K in {8,4,2,1}); one 256B src
    descriptor serves K edges (hu broadcast via step-0 AP). dst side stays
    one 256B descriptor per edge. Every gather is split into 128-aligned
    pieces of <=2048 descriptors spread greedily over the queues, so the
    per-queue serial quantum stays <=16us and the 4 Q7 pairs stay packed;
    8192-edge chunks keep the instruction count (and its ~0.6us/instruction
    fixed cost) low.
  - int16 gather indices => 4-way range bucketing (src>=32768, dst>=32768)
    with per-range base pointers; host permutes edges, unpermutes results.
  - DVE: hu broadcast across K members via step-0 AP, in-place multiply
    into the hv tile, segment-reduce 64-feature dim to one score per edge.
  - Output [128, tiles] stored contiguously; host transposes + scatters.
"""

import os
from contextlib import ExitStack

import numpy as np

import concourse.bacc as bacc
import concourse.mybir as mybir
from concourse import library_config
from concourse.bass import AP
from concourse._compat import get_trn_type
from concourse.bass_utils import run_bass_kernel_spmd

N_NODES = 50000
NPAD = 50008  # h padded so reads past the last node stay in bounds
D = 64
P = 128
N_CORES = 8
SPLIT = 32768
NQ = 4  # SWDGE queues (each with its own Q7 descriptor-gen cpu pair)
NB = 5  # buffer pairs

G_MAP = {8: 1024, 4: 2048, 2: 4096, 1: 2048}  # units per chunk (small K=1 chunks end the schedule with a short DVE tail)

TRACE = False
LAST_RESULT = None


def _ensure_ntff_hook():
    """bass_utils' trace path imports antenv.axon_hooks, which this image's
    antenv package lacks. Recreate it from the boot helper so trace=True
    works; harmless no-op if the real module exists."""
    import sys
    import types

    try:
        import antenv.axon_hooks  # noqa: F401

        return
    except ImportError:
        pass
    try:
        import antenv
        from trn_agent_boot.trn_boot import _ntff_profile_via_ctypes

        hook = _ntff_profile_via_ctypes("/opt/axon/libaxon_pjrt.so")
        m = types.ModuleType("antenv.axon_hooks")
        m.get_axon_ntff_profile_hook = lambda: hook
        m.set_axon_ntff_profile_hook = lambda h: None
        sys.modules["antenv.axon_hooks"] = m
        antenv.axon_hooks = m
    except Exception:
        pass


def _wrap_idx(vals):
    """int16 index array [Npc] -> the [128, Npc/16] SBUF layout dma_gather
    expects (idx i at partition i%16, column i//16, replicated over the 8
    groups of 16 partitions — each SWDGE queue's Q7 pair reads its own
    group)."""
    w = vals.reshape(-1, 16).T  # [16, Npc/16]
    return np.ascontiguousarray(np.tile(w, (8, 1)))  # [128, Npc/16]


def _host_prep(src, dst):
    """Sort by (range-group, src); decompose equal-src runs into K-units.

    Returns (schedule, seqs, sidx_per_core, didx_per_core, u_total, e_total):
      schedule: list of (K, s_hi, d_hi, u_off, e_off, n_units), same all cores
      seqs: [N_CORES, e_total] global edge id per output position (-1 pad)
    """
    E = src.shape[0]
    g = (src >= SPLIT).astype(np.int8) * 2 + (dst >= SPLIT).astype(np.int8)
    order0 = np.lexsort((src, g))
    sg, ss, sd = g[order0], src[order0], dst[order0]

    new = np.ones(E, bool)
    new[1:] = (sg[1:] != sg[:-1]) | (ss[1:] != ss[:-1])
    run_start = np.flatnonzero(new)
    d = np.diff(np.append(run_start, E))
    run_id = np.cumsum(new) - 1
    r = np.arange(E) - run_start[run_id]
    dd = d[run_id]
    n8 = (dd // 8) * 8
    n4 = n8 + (((dd - n8) // 4) * 4)
    n2 = n4 + (((dd - n4) // 2) * 2)
    K_e = np.where(r < n8, 8, np.where(r < n4, 4, np.where(r < n2, 2, 1)))
    m_e = np.where(
        K_e == 8, r % 8,
        np.where(K_e == 4, (r - n8) % 4, np.where(K_e == 2, (r - n4) % 2, 0)),
    )
    first = m_e == 0

    pad_units = N_CORES * P
    schedule = []
    sidx_parts = [[] for _ in range(N_CORES)]
    didx_parts = [[] for _ in range(N_CORES)]
    seq_parts = [[] for _ in range(N_CORES)]
    u_off = 0
    e_off = 0
    for K in (8, 4, 2, 1):
        for gg in range(4):
            starts = np.flatnonzero(first & (K_e == K) & (sg == gg))
            if starts.size == 0:
                continue
            Upad = -(-starts.size // pad_units) * pad_units
            buf = np.full(Upad, -1, dtype=np.int64)
            buf[: starts.size] = starts
            U = Upad // N_CORES  # per-core units, multiple of 128
            s_hi, d_hi = gg >= 2, gg % 2 == 1
            for c in range(N_CORES):
                uc = buf[c * U : (c + 1) * U]
                valid = uc >= 0
                sv = np.zeros(U, np.int64)
                sv[valid] = ss[uc[valid]] - (SPLIT if s_hi else 0)
                sidx_parts[c].append(sv.astype(np.int16))
                dvals = np.zeros(U * K, np.int64)
                ids = np.full(U * K, -1, np.int64)
                uu = np.arange(U)
                for m in range(K):
                    pos = (K * (uu // P) + m) * P + uu % P
                    dvals[pos[valid]] = sd[uc[valid] + m] - (
                        SPLIT if d_hi else 0
                    )
                    ids[pos[valid]] = order0[uc[valid] + m]
                didx_parts[c].append(dvals.astype(np.int16))
                seq_parts[c].append(ids)
            # chunks
            o, rem = 0, U
            Gn = G_MAP[K]
            while rem > 0:
                n = min(Gn, rem)
                schedule.append((K, s_hi, d_hi, u_off + o, e_off + o * K, n))
                o += n
                rem -= n
            u_off += U
            e_off += U * K

    seqs = np.stack([np.concatenate(p) for p in seq_parts])
    sidx = [np.concatenate(p) for p in sidx_parts]
    didx = [np.concatenate(p) for p in didx_parts]
    return schedule, seqs, sidx, didx, u_off, e_off


def _build_nc(schedule, u_total, e_total):
    SCOLS = u_total // 16
    DCOLS = e_total // 16
    TILES = e_total // P

    nc = bacc.Bacc(
        get_trn_type() or "TRN2",
        debug=False,
        dynamic_dma_scratch_size=32768,
        num_swdge_queues=NQ,
    )
    h = nc.dram_tensor("h", [NPAD, D], mybir.dt.float32, kind="ExternalInput")
    sidx = nc.dram_tensor("sidx", [P, SCOLS], mybir.dt.int16, kind="ExternalInput")
    didx = nc.dram_tensor("didx", [P, DCOLS], mybir.dt.int16, kind="ExternalInput")
    out = nc.dram_tensor("out", [P, TILES], mybir.dt.float32, kind="ExternalOutput")

    # per-row base pointers for the two int16 index ranges
    h_lo = h[0:SPLIT, :]
    h_hi = h[SPLIT:NPAD, :]
    nch = len(schedule)

    # split each gather into 128-aligned pieces of <=2048 descriptors
    # (quanta small enough to pack the 4 Q7 pairs; pieces below 512 are
    # pathologically slow, so remainders fold into the last piece)
    def pieces(size):
        if size >= 4096:
            out = []
            rem = size
            while rem > 2048 + 1024:
                out.append(2048)
                rem -= 2048
            if rem > 2048:
                h = (rem // 2 // P) * P
                out += [h, rem - h]
            else:
                out.append(rem)
            return out
        if size >= 1024 and (size // 2) % P == 0:
            return [size // 2, size - size // 2]
        return [size]

    # greedy queue assignment balancing descriptor counts per piece. Pieces
    # covering the first half of a chunk are tagged half-A (separate gather
    # semaphore) so the DVE can start on a chunk's first half while its
    # second half is still gathering. The gather stream itself (issue order,
    # queue choice) is identical to the untagged version.
    def prefix_half(lst, half):
        s = 0
        for i, sz in enumerate(lst):
            s += sz
            if s == half:
                return i + 1
            if s > half:
                return 0
        return 0

    qloads = [0] * NQ
    qassign = []
    for (K, s_hi, d_hi, uo, eo, n) in schedule:
        ne = n * K
        sp_raw = pieces(n)
        dp_raw = pieces(ne)
        ks = prefix_half(sp_raw, n // 2) if n % 256 == 0 else 0
        kd = prefix_half(dp_raw, ne // 2) if ne % 256 == 0 else 0
        split = bool(ks and kd)
        sps, dps = [], []
        for i, sz in enumerate(sp_raw):
            q = min(range(NQ), key=lambda x: qloads[x])
            qloads[q] += sz
            sps.append((q, sz, split and i < ks))
        for i, sz in enumerate(dp_raw):
            q = min(range(NQ), key=lambda x: qloads[x])
            qloads[q] += sz
            dps.append((q, sz, split and i < kd))
        qassign.append((sps, dps, split))

    with ExitStack() as stack:
        ent = stack.enter_context
        hu = [ent(nc.sbuf_tensor(f"hu{i}", [P, 2048], mybir.dt.float32)) for i in range(NB)]
        hv = [ent(nc.sbuf_tensor(f"hv{i}", [P, 4096], mybir.dt.float32)) for i in range(NB)]
        sidx_sb = ent(nc.sbuf_tensor("sidx_sb", [P, SCOLS], mybir.dt.int16))
        didx_sb = ent(nc.sbuf_tensor("didx_sb", [P, DCOLS], mybir.dt.int16))
        outb = ent(nc.sbuf_tensor("outb", [P, TILES], mybir.dt.float32))
        io = ent(nc.semaphore("io"))
        io2 = ent(nc.semaphore("io2"))
        gsem = [ent(nc.semaphore(f"g{i}")) for i in range(NB)]
        gsa = [ent(nc.semaphore(f"a{i}")) for i in range(NB)]
        vsem = [ent(nc.semaphore(f"v{i}")) for i in range(NB)]
        mr = ent(nc.semaphore("mr"))

        def hu_ap(b, t_u, off=0):
            base = hu[b][:]
            return AP(base.tensor, off * D, [[2048, P], [D, t_u], [1, D]])

        def hu_bcast(b, t_u, K, off=0):
            base = hu[b][:]
            return AP(base.tensor, off * D, [[2048, P], [D, t_u], [0, K], [1, D]])

        def hv_ap(b, t_e, off=0):
            base = hv[b][:]
            return AP(base.tensor, off * D, [[4096, P], [D, t_e], [1, D]])

        def hv_4d(b, t_u, K, off=0):
            base = hv[b][:]
            return AP(base.tensor, off * D * K, [[4096, P], [D * K, t_u], [D, K], [1, D]])

        with nc.Block() as block:

            @block.sync
            def _(sync):
                sync.dma_start(sidx_sb[:], sidx[:]).then_inc(io, 16)
                sync.dma_start(didx_sb[:], didx[:]).then_inc(io, 16)
                # store the output round by round so only the last ~1/5 of it
                # remains after the final reduce
                nrounds = -(-nch // NB)
                for r in range(nrounds):
                    c_lo, c_hi = r * NB, min(nch, (r + 1) * NB)
                    for c in range(c_lo, c_hi):
                        sync.wait_ge(vsem[c % NB], c // NB + 1)
                    eo_lo = schedule[c_lo][4]
                    eo_hi = schedule[c_hi][4] if c_hi < nch else e_total
                    sync.dma_start(
                        out[:, eo_lo // P : eo_hi // P],
                        outb[:, eo_lo // P : eo_hi // P],
                    ).then_inc(io2, 16)
                sync.wait_ge(io2, 16 * nrounds)

            @block.gpsimd
            def _(gp):
                gp.load_library(library_config.mlp)
                gp.wait_ge(io, 32)
                for c, (K, s_hi, d_hi, uo, eo, n) in enumerate(schedule):
                    b = c % NB
                    sps, dps, split = qassign[c]
                    if c >= NB:
                        gp.wait_ge(vsem[b], c // NB)
                    off = 0
                    for q, sz, isA in sps:
                        gp.dma_gather(
                            hu_ap(b, sz // P, off=off // P),
                            h_hi if s_hi else h_lo,
                            sidx_sb[:, (uo + off) // 16 : (uo + off + sz) // 16],
                            sz,
                            sz,
                            D,
                            single_packet=False,
                            queue_num=q,
                        ).then_inc(gsa[b] if isA else gsem[b], 16)
                        off += sz
                    off = 0
                    for q, sz, isA in dps:
                        gp.dma_gather(
                            hv_ap(b, sz // P, off=off // P),
                            h_hi if d_hi else h_lo,
                            didx_sb[:, (eo + off) // 16 : (eo + off + sz) // 16],
                            sz,
                            sz,
                            D,
                            single_packet=False,
                            queue_num=q,
                        ).then_inc(gsa[b] if isA else gsem[b], 16)
                        off += sz

            @block.vector
            def _(ve):
                ga_acc, gb_acc = {}, {}
                ga_needed, gb_needed, m_idx = [], [], []
                m_acc = 0
                for c in range(nch):
                    bb = c % NB
                    sps, dps, split = qassign[c]
                    nA = sum(1 for _, _, a in sps + dps if a)
                    nB = len(sps) + len(dps) - nA
                    ga_acc[bb] = ga_acc.get(bb, 0) + 16 * nA
                    gb_acc[bb] = gb_acc.get(bb, 0) + 16 * nB
                    ga_needed.append(ga_acc[bb])
                    gb_needed.append(gb_acc[bb])
                    if split:
                        m_idx.append((m_acc + 1, m_acc + 2))
                        m_acc += 2
                    else:
                        m_idx.append((None, m_acc + 1))
                        m_acc += 1
                for c, (K, s_hi, d_hi, uo, eo, n) in enumerate(schedule):
                    b = c % NB
                    sps, dps, split = qassign[c]
                    t_u = n // P
                    t_e = t_u * K
                    ma, mb = m_idx[c]
                    halves = ((0, True), (1, False)) if split else ((0, False),)
                    for hi, is_half in halves:
                        t2 = t_u // 2 if split else t_u
                        uoff = hi * t2
                        if hi == 0 and split:
                            ve.wait_ge(gsa[b], ga_needed[c])
                        else:
                            ve.wait_ge(gsem[b], gb_needed[c])
                        if K == 1:
                            prod_in1 = hu_ap(b, t2, off=uoff)
                            prod = hv_ap(b, t2, off=uoff)
                        else:
                            prod_in1 = hu_bcast(b, t2, K, off=uoff)
                            prod = hv_4d(b, t2, K, off=uoff)
                        ve.tensor_tensor(
                            out=prod, in0=prod, in1=prod_in1,
                            op=mybir.AluOpType.mult,
                        ).then_inc(mr, 1)
                        ve.wait_ge(mr, ma if (hi == 0 and split) else mb)
                        red = ve.tensor_reduce(
                            out=outb[:, eo // P + uoff * K : eo // P + uoff * K + t2 * K],
                            in_=prod,
                            axis=mybir.AxisListType.X,
                            op=mybir.AluOpType.add,
                        )
                        if hi == (1 if split else 0):
                            red.then_inc(vsem[b], 1)

    nc.compile()
    return nc


def kernel(h, src, dst):
    global LAST_RESULT
    h = np.asarray(h, dtype=np.float32)
    hp = np.zeros((NPAD, D), np.float32)
    hp[:N_NODES] = h
    src = np.asarray(src).astype(np.int64)
    dst = np.asarray(dst).astype(np.int64)
    E = src.shape[0]

    schedule, seqs, sidx, didx, u_total, e_total = _host_prep(src, dst)
    in_maps = [
        {"h": hp, "sidx": _wrap_idx(sidx[c]), "didx": _wrap_idx(didx[c])}
        for c in range(N_CORES)
    ]
    nc = _build_nc(schedule, u_total, e_total)

    if TRACE or os.environ.get("BASS_TRACE"):
        _ensure_ntff_hook()
    res = run_bass_kernel_spmd(nc, in_maps, core_ids=list(range(N_CORES)), trace=TRACE)
    LAST_RESULT = res

    out = np.empty(E, np.float32)
    for c in range(N_CORES):
        dots = res.results[c]["out"].T.reshape(-1)
        seq = seqs[c]
        valid = seq >= 0
        out[seq[valid]] = dots[valid]
    return out
